# revision 26
# baseline (speedup 1.0000x reference)
"""Trainium2 Bass kernel for a 3-layer TransformerConv GNN (nn_EncoderTransformerConv).

v2 strategy (8 NeuronCores, SPMD):
  - Nodes partitioned across cores (6250 real/core, padded 6272 = 49 blocks of
    128); edges partitioned by dst core, grouped into (block-pack, src-half,
    dst-block) tiles of 128 edges.
  - Per-edge work gathers RAW source features (x for layer 1, h for layers 2/3;
    256B rows) instead of projected k|v. The k/v projections are folded through
    the linearity:
      alpha_e = <q[dst] Wk^T, f[src]>/8 (+ per-dst shift, dropped: softmax-inv)
      out[d]  = sum_h (sum_e attn_e f[src]) @ (Wv_h/H) + skip
    so phase A computes only OWN-node q~ = f @ (Wq Wk^T)/8 and s = f @ Ws; no
    kv table is materialized at all.
  - dma_gather descriptor generation is spread over 4 SWDGE queues (core pairs)
    -> ~4x Q7 descgen throughput.
  - One-hot scatter/broadcast matrices S/ST are fp8 (exact); matmuls mix fp8
    one-hots with bf16 data.
  - Per dst block: PSUM accumulates [ex | ex (x) f_src] reductions transposed
    (aggT[(h,c), node]) across both src halves (block-major pack ordering), then
    the epilogue applies @Wv/H per head, 1/(den+eps), skip and relu.
  - One AllGather per layer publishes row-major h (padded to 256B rows) as the
    next layer's gather table.
"""
import os
import sys

sys.path.insert(0, "/opt/trn_rl_repo")

import ml_dtypes
import numpy as np

import concourse.bass as bass
import concourse.bacc as bacc
import concourse.mybir as mybir
import concourse.tile as tile
from concourse import bass_utils, library_config
from concourse.masks import make_identity

F32 = mybir.dt.float32
BF16 = mybir.dt.bfloat16
FP8 = mybir.dt.float8e4
I16 = mybir.dt.int16
AF = mybir.ActivationFunctionType
OP = mybir.AluOpType
BNP = ml_dtypes.bfloat16
F8NP = ml_dtypes.float8_e4m3

SPEC = dict(N=50000, E=800000, D_IN=128, HID=64, H=2, M=8)
TILE_E = 128
CHUNK_T = 12
BLK = 128
PACK_B = 1            # dst blocks per pack (psum group spans both halves)
LDIM = {1: 128, 2: 64, 3: 64}   # per-layer input feature dim


def _derive(cfg):
    d = dict(cfg)
    d["C"] = d["HID"]
    d["NPC_REAL"] = d["N"] // d["M"]
    d["NBLK"] = -(-d["NPC_REAL"] // BLK)
    d["NPC"] = d["NBLK"] * BLK
    d["NPAD"] = d["M"] * d["NPC"]
    d["HALF"] = d["NPAD"] // 2
    pa = 1
    for c in range(1, 9):
        if d["NBLK"] % c == 0:
            pa = c
    d["PA_CHUNK"] = pa
    return d


def _wrap_idx(a):
    Mn, n = a.shape
    w = a.reshape(Mn, n // 16, 16).transpose(0, 2, 1)
    return np.ascontiguousarray(np.tile(w, (1, 8, 1))).astype(np.int16)


def _prep(x, edge_index, d):
    M, NPC_REAL, NPC, NPAD, HALF, NBLK = (
        d["M"], d["NPC_REAL"], d["NPC"], d["NPAD"], d["HALF"], d["NBLK"])
    N, D_IN = d["N"], d["D_IN"]

    src = np.asarray(edge_index[0]).astype(np.int64)
    dst = np.asarray(edge_index[1]).astype(np.int64)
    core = dst // NPC_REAL
    dst_l = dst - core * NPC_REAL
    blk = dst_l // BLK
    src_p = (src // NPC_REAL) * NPC + (src % NPC_REAL)
    half = (src_p >= HALF).astype(np.int64)

    counts = np.zeros((M, 2, NBLK), np.int64)
    np.add.at(counts, (core, half, blk), 1)
    tiles = np.maximum(1, -(-counts.max(axis=0) // TILE_E))  # [2, NBLK]

    # group enumeration: packs of PACK_B blocks; within a pack halves 0 then 1
    packs = [list(range(p, min(p + PACK_B, NBLK)))
             for p in range(0, NBLK, PACK_B)]
    order_groups = []
    gidx = np.zeros((2, NBLK), np.int64)
    for pack in packs:
        for f in (0, 1):
            for b in pack:
                gidx[f, b] = len(order_groups)
                order_groups.append((f, b))
    NG = len(order_groups)
    tpg = np.array([tiles[f, b] for (f, b) in order_groups])
    tile_off = np.concatenate([[0], np.cumsum(tpg)])
    TT = int(tile_off[-1])

    key = core * NG + gidx[half, blk]
    order = np.argsort(key, kind="stable")
    sk = key[order]
    new_run = np.ones(len(sk), bool)
    new_run[1:] = sk[1:] != sk[:-1]
    run_idx = np.cumsum(new_run) - 1
    starts = np.nonzero(new_run)[0]
    rank = np.arange(len(sk)) - starts[run_idx]
    grp = gidx[half, blk][order]
    pos = tile_off[grp] * TILE_E + rank
    corev = core[order]

    kv_idx = np.zeros((M, TT * TILE_E), np.int64)
    dloc = np.full((M, TT * TILE_E), -1, np.int64)
    kv_idx[corev, pos] = (src_p - half * HALF)[order]
    dloc[corev, pos] = (dst_l - blk * BLK)[order]
    assert kv_idx.max() < 2 ** 15

    S = np.zeros((M, 128, TT * BLK), F8NP)
    ST = np.zeros((M, 128, TT * BLK), F8NP)
    dd = dloc.reshape(M, TT, TILE_E)
    mm, tt, pp = np.nonzero(dd >= 0)
    dv = dd[mm, tt, pp]
    S[mm, pp, tt * BLK + dv] = 1.0
    ST[mm, dv, tt * BLK + pp] = 1.0
    kv_w = _wrap_idx(kv_idx)

    # node features: own-transposed and padded row-major table
    xT_pad = np.zeros((D_IN, NPAD), np.float32)
    n_ids = np.arange(N)
    pid = (n_ids // NPC_REAL) * NPC + (n_ids % NPC_REAL)
    xT_pad[:, pid] = np.asarray(x).T
    xT = xT_pad.astype(BNP)
    xrow = np.ascontiguousarray(xT.T)          # [NPAD, 128] bf16

    # per-tile meta: (f, b, start, stop)
    meta = []
    for (f, b), T in zip(order_groups, tpg):
        for i in range(int(T)):
            meta.append((f, b, f == 0 and i == 0, f == 1 and i == int(T) - 1))
    # chunks: within each (pack, f) run, greedy <= CHUNK_T tiles
    chunks = []
    gi = 0
    for pack in packs:
        for f in (0, 1):
            nt_run = int(sum(tiles[f, b] for b in pack))
            t0 = int(tile_off[gi])
            while nt_run > 0:
                nt = min(nt_run, CHUNK_T)
                chunks.append((t0, nt, f))
                t0 += nt
                nt_run -= nt
            gi += len(pack)

    in_maps = []
    for m in range(M):
        im = dict(
            xoT=np.ascontiguousarray(xT[:, m * NPC:(m + 1) * NPC]),
            xrow=xrow,
            kvidx=np.ascontiguousarray(kv_w[m]),
            S_in=np.ascontiguousarray(S[m]),
            ST_in=np.ascontiguousarray(ST[m]),
        )
        in_maps.append(im)
    return in_maps, dict(TT=TT, meta=meta, chunks=chunks, tiles=tiles)


def _host_weights(inputs, d):
    """Per-layer folded weights -> dict of input arrays (same for all cores)."""
    H = d["H"]
    out = {}
    for L in (1, 2, 3):
        dl = LDIM[L]
        Wq = np.asarray(inputs[f"W{L}q"], np.float32)
        bq = np.asarray(inputs[f"b{L}q"], np.float32)
        Wk = np.asarray(inputs[f"W{L}k"], np.float32)
        bk = np.asarray(inputs[f"b{L}k"], np.float32)
        Wv = np.asarray(inputs[f"W{L}v"], np.float32)
        bv = np.asarray(inputs[f"b{L}v"], np.float32)
        Ws = np.asarray(inputs[f"W{L}s"], np.float32)
        bs = np.asarray(inputs[f"b{L}s"], np.float32)
        QW = 2 * dl
        WA = np.zeros((dl, QW + 64), np.float32)
        bA = np.zeros(QW + 64, np.float32)
        scale = 1.0 / np.sqrt(np.float32(d["C"]))
        for h in range(H):
            Wq_h = Wq[:, h * 64:(h + 1) * 64]
            Wk_h = Wk[:, h * 64:(h + 1) * 64]
            WA[:, h * dl:(h + 1) * dl] = (Wq_h @ Wk_h.T) * scale
            bA[h * dl:(h + 1) * dl] = (bq[h * 64:(h + 1) * 64] @ Wk_h.T) * scale
        WA[:, QW:] = Ws
        bA[QW:] = bs + (bv[0:64] + bv[64:128]) / H
        Wv2 = np.zeros((dl, 128), np.float32)
        for h in range(H):
            Wv2[:, h * 64:(h + 1) * 64] = Wv[:, h * 64:(h + 1) * 64] / H
        if L == 1:
            out["WA1"] = WA.astype(BNP)
            out["bArep1"] = np.ascontiguousarray(
                np.tile(bA[None, :], (128, 1)).astype(np.float32))
        else:
            out[f"WA{L}"] = np.concatenate([WA, bA[None, :]], 0).astype(BNP)
        out[f"Wv2{L}"] = Wv2.astype(BNP)
    return out


def build_module(d, meta):
    TT, chunks, tmeta = meta["TT"], meta["chunks"], meta["meta"]
    M, NPC, NPAD, HALF, NBLK, PA_CHUNK = (
        d["M"], d["NPC"], d["NPAD"], d["HALF"], d["NBLK"], d["PA_CHUNK"])
    D_IN, HID, H = d["D_IN"], d["HID"], d["H"]
    RANK_CH = NBLK // PA_CHUNK

    nc = bacc.Bacc("TRN2", target_bir_lowering=False, debug=False,
                   num_devices=M, num_swdge_queues=4)
    inp = {}
    for name, shape, dt in [
        ("xoT", [D_IN, NPC], BF16), ("xrow", [NPAD, 128], BF16),
        ("WA1", [128, 320], BF16), ("bArep1", [128, 320], F32),
        ("WA2", [65, 192], BF16), ("WA3", [65, 192], BF16),
        ("Wv21", [128, 128], BF16), ("Wv22", [64, 128], BF16),
        ("Wv23", [64, 128], BF16),
        ("kvidx", [128, TT * 8], I16),
        ("S_in", [128, TT * BLK], FP8), ("ST_in", [128, TT * BLK], FP8),
    ]:
        inp[name] = nc.dram_tensor(name, shape, dt, kind="ExternalInput")
    h_out = nc.dram_tensor("h_out", [NPC, HID], F32, kind="ExternalOutput")

    with tile.TileContext(nc) as tc:
        with tc.tile_pool(name="dram", bufs=1, space="DRAM") as dram, \
             tc.tile_pool(name="res", bufs=1) as res:
            coll_in = dram.tile([NPC, 128], BF16)
            coll_out = dram.tile([NPAD, 128], BF16)

            nc.gpsimd.load_library(library_config.mlp)

            WA1_sb = res.tile([128, 320], BF16)
            bArep1_sb = res.tile([128, 320], F32)
            WA2_sb = res.tile([65, 192], BF16)
            WA3_sb = res.tile([65, 192], BF16)
            Wv2_sb = {1: res.tile([128, 128], BF16, name="Wv2_1"),
                      2: res.tile([64, 128], BF16, name="Wv2_2"),
                      3: res.tile([64, 128], BF16, name="Wv2_3")}
            kvidx_sb = res.tile([128, TT * 8], I16)
            hTown = res.tile([HID + 1, NPC], BF16)
            ident = res.tile([128, 128], BF16)
            epsT = res.tile([128, H], F32)

            for sb, t in ((WA1_sb, "WA1"), (bArep1_sb, "bArep1"),
                          (WA2_sb, "WA2"), (WA3_sb, "WA3"),
                          (Wv2_sb[1], "Wv21"), (Wv2_sb[2], "Wv22"),
                          (Wv2_sb[3], "Wv23"), (kvidx_sb, "kvidx")):
                nc.sync.dma_start(sb[:], inp[t].ap())
            make_identity(nc, ident[:])
            nc.vector.memset(hTown[HID:HID + 1, :], 1.0)
            nc.vector.memset(epsT[:], 1e-16)

            for layer in (1, 2, 3):
                dl = LDIM[layer]
                QW = 2 * dl
                RH = H + QW

                with tc.tile_pool(name="ly", bufs=1) as ly, \
                     tc.tile_pool(name="pa", bufs=2) as pa:
                    qf_sb = ly.tile([128, NBLK * QW], BF16)
                    s_sb = ly.tile([128, NBLK * HID], F32)

                    # ---------- Phase A: own-node q~ and s ----------
                    with tc.tile_pool(name="pap", bufs=4, space="PSUM") as pap:
                        if layer == 1:
                            for ch in range(RANK_CH):
                                cols = slice(ch * PA_CHUNK * 128,
                                             (ch + 1) * PA_CHUNK * 128)
                                la = pa.tile([D_IN, PA_CHUNK * 128], BF16,
                                             tag="la")
                                nc.sync.dma_start(la[:],
                                                  inp["xoT"].ap()[:, cols])
                                for t in range(PA_CHUNK):
                                    gt = ch * PA_CHUNK + t
                                    ps = pap.tile([128, 320], F32, name="psA",
                                                  tag="psA")
                                    nc.tensor.matmul(
                                        ps[:], la[:, t * 128:(t + 1) * 128],
                                        WA1_sb[:], start=True, stop=True)
                                    nc.vector.tensor_tensor(
                                        qf_sb[:, gt * QW:(gt + 1) * QW],
                                        ps[:, 0:QW], bArep1_sb[:, 0:QW],
                                        op=OP.add)
                                    nc.vector.tensor_tensor(
                                        s_sb[:, gt * HID:(gt + 1) * HID],
                                        ps[:, QW:QW + HID],
                                        bArep1_sb[:, QW:QW + HID], op=OP.add)
                        else:
                            WA_sb = WA2_sb if layer == 2 else WA3_sb
                            for gt in range(NBLK):
                                ps = pap.tile([128, 192], F32, name="psA",
                                              tag="psA")
                                nc.tensor.matmul(
                                    ps[:], hTown[:, gt * 128:(gt + 1) * 128],
                                    WA_sb[:], start=True, stop=True)
                                nc.scalar.copy(qf_sb[:, gt * QW:(gt + 1) * QW],
                                               ps[:, 0:QW])
                                nc.vector.tensor_copy(
                                    s_sb[:, gt * HID:(gt + 1) * HID],
                                    ps[:, QW:QW + HID])

                    # ---------- Phase B: edges ----------
                    with tc.tile_pool(name="pb", bufs=3) as pb, \
                         tc.tile_pool(name="pb1", bufs=4) as pb1, \
                         tc.tile_pool(name="pbp", bufs=2, space="PSUM") as pbp, \
                         tc.tile_pool(name="qep", bufs=2, space="PSUM") as qep, \
                         tc.tile_pool(name="epp", bufs=1, space="PSUM") as epp, \
                         tc.tile_pool(name="ep", bufs=2) as ep:
                        psum_blk = {}
                        for j, (t0, nt, fhalf) in enumerate(chunks):
                            n = nt * TILE_E
                            G = pb.tile([128, CHUNK_T, 128], BF16, tag="G",
                                        bufs=10)
                            Sg = pb1.tile([128, CHUNK_T * BLK], FP8, tag="Sg",
                                          bufs=6)
                            STg = pb1.tile([128, CHUNK_T * BLK], FP8,
                                           tag="STg", bufs=6)
                            prod = pb1.tile([128, CHUNK_T * QW], BF16,
                                            tag="prod", bufs=2)
                            alph = pb1.tile([128, CHUNK_T * H], F32, tag="alph",
                                            bufs=2)
                            rhs = pb.tile([128, CHUNK_T, RH], BF16, tag="rhs")

                            if layer == 1:
                                in_ap = inp["xrow"].ap()[
                                    fhalf * HALF:(fhalf + 1) * HALF, :]
                            else:
                                in_ap = coll_out[
                                    fhalf * HALF:(fhalf + 1) * HALF, :]
                            nc.gpsimd.dma_gather(
                                out_ap=G[:, 0:nt, :], in_ap=in_ap,
                                idxs_ap=kvidx_sb[:, t0 * 8:t0 * 8 + nt * 8],
                                num_idxs=n, num_idxs_reg=n, elem_size=128,
                                single_packet=False, queue_num=j % 4)
                            nc.sync.dma_start(
                                Sg[:, 0:n],
                                inp["S_in"].ap()[:, t0 * BLK:t0 * BLK + n])
                            nc.sync.dma_start(
                                STg[:, 0:n],
                                inp["ST_in"].ap()[:, t0 * BLK:t0 * BLK + n])

                            for i in range(nt):
                                b = tmeta[t0 + i][1]
                                qe = qep.tile([128, QW], F32, name="qe", tag="qe")
                                nc.tensor.matmul(
                                    qe[:], STg[:, i * BLK:(i + 1) * BLK],
                                    qf_sb[:, b * QW:(b + 1) * QW],
                                    start=True, stop=True)
                                nc.vector.tensor_tensor(
                                    out=prod[:, i * QW:(i + 1) * QW].rearrange(
                                        "p (h c) -> p h c", c=dl),
                                    in0=qe[:].rearrange("p (h c) -> p h c", c=dl),
                                    in1=G[:, i, 0:dl].rearrange(
                                        "p (o c) -> p o c", o=1
                                    ).to_broadcast([128, H, dl]),
                                    op=OP.mult)
                            nc.vector.reduce_sum(
                                alph[:, 0:nt * H].rearrange(
                                    "p (t h) -> p t h", h=H),
                                prod[:, 0:nt * QW].rearrange(
                                    "p (t h c) -> p t h c", h=H, c=dl),
                                axis=mybir.AxisListType.X)
                            nc.scalar.activation(
                                rhs[:, 0:nt, 0:H],
                                alph[:, 0:nt * H].rearrange(
                                    "p (t h) -> p t h", h=H),
                                AF.Exp)
                            nc.vector.tensor_tensor(
                                out=rhs[:, 0:nt, H:RH].rearrange(
                                    "p t (h c) -> p t h c", c=dl),
                                in0=G[:, 0:nt, 0:dl].rearrange(
                                    "p t (o c) -> p t o c", o=1
                                ).to_broadcast([128, nt, H, dl]),
                                in1=rhs[:, 0:nt, 0:H].to_broadcast(
                                    [128, nt, H, dl]),
                                op=OP.mult)

                            for i in range(nt):
                                f, b, st, sp = tmeta[t0 + i]
                                if st:
                                    if layer == 1:
                                        psum_blk[b] = (
                                            pbp.tile([128, dl], F32,
                                                     name="agg0", tag="agg0"),
                                            pbp.tile([128, dl], F32,
                                                     name="agg1", tag="agg1",
                                                     bufs=1),
                                            pbp.tile([128, H], F32,
                                                     name="den", tag="den",
                                                     bufs=1))
                                    else:
                                        psum_blk[b] = (
                                            pbp.tile([128, QW], F32,
                                                     name="agg", tag="agg"),
                                            pbp.tile([128, H], F32,
                                                     name="den", tag="den",
                                                     bufs=2))
                                tiles_b = psum_blk[b]
                                den_ps = tiles_b[-1]
                                if layer == 1:
                                    for h in range(H):
                                        nc.tensor.matmul(
                                            tiles_b[h][:],
                                            rhs[:, i,
                                                H + h * dl:H + (h + 1) * dl],
                                            Sg[:, i * BLK:(i + 1) * BLK],
                                            start=st, stop=sp)
                                else:
                                    nc.tensor.matmul(
                                        tiles_b[0][:],
                                        rhs[:, i, H:H + QW],
                                        Sg[:, i * BLK:(i + 1) * BLK],
                                        start=st, stop=sp)
                                nc.tensor.matmul(
                                    den_ps[:],
                                    Sg[:, i * BLK:(i + 1) * BLK],
                                    rhs[:, i, 0:H], start=st, stop=sp)
                                if not sp:
                                    continue
                                # ---- epilogue for block b ----
                                tiles_b = psum_blk.pop(b)
                                den_ps = tiles_b[-1]
                                naT = []
                                for h in range(H):
                                    nt_h = ep.tile([dl, 128], BF16,
                                                   tag=f"naT{h}",
                                                   name=f"naT{h}")
                                    if layer == 1:
                                        nc.scalar.copy(nt_h[:],
                                                       tiles_b[h][:])
                                    else:
                                        nc.scalar.copy(
                                            nt_h[:],
                                            tiles_b[0][h * dl:(h + 1) * dl, :])
                                    naT.append(nt_h)
                                rec = ep.tile([128, H], F32, tag="rec")
                                nc.scalar.copy(rec[:], den_ps[:])
                                nc.vector.scalar_tensor_tensor(
                                    out=rec[:], in0=rec[:], scalar=1.0,
                                    in1=epsT[:], op0=OP.mult, op1=OP.add)
                                nc.vector.reciprocal(rec[:], rec[:])
                                P = epp.tile([128, 128], F32, name="P", tag="P")
                                nc.tensor.matmul(
                                    P[:, 0:64], naT[0][:],
                                    Wv2_sb[layer][:, 0:64],
                                    start=True, stop=True)
                                nc.tensor.matmul(
                                    P[:, 64:128], naT[1][:],
                                    Wv2_sb[layer][:, 64:128],
                                    start=True, stop=True)
                                m0 = ep.tile([128, HID], F32, tag="m0")
                                nc.vector.scalar_tensor_tensor(
                                    out=m0[:], in0=P[:, 0:64],
                                    scalar=rec[:, 0:1],
                                    in1=s_sb[:, b * HID:(b + 1) * HID],
                                    op0=OP.mult, op1=OP.add)
                                hp2 = ep.tile([128, HID], F32, tag="hp2")
                                nc.vector.scalar_tensor_tensor(
                                    out=hp2[:], in0=P[:, 64:128],
                                    scalar=rec[:, 1:2], in1=m0[:],
                                    op0=OP.mult, op1=OP.add)
                                if layer < 3:
                                    hblk = ep.tile([128, HID], BF16, tag="hblk")
                                    nc.scalar.activation(hblk[:], hp2[:],
                                                         AF.Relu)
                                    pst = epp.tile([HID, 128], BF16, tag="pst")
                                    nc.tensor.transpose(pst[:], hblk[:],
                                                        ident[:])
                                    nc.vector.tensor_copy(
                                        hTown[0:HID, b * 128:(b + 1) * 128],
                                        pst[:])
                                    nc.sync.dma_start(
                                        coll_in[b * 128:(b + 1) * 128, 0:HID],
                                        hblk[:])
                                else:
                                    hblk = ep.tile([128, HID], F32, tag="hblkf")
                                    nc.scalar.activation(hblk[:], hp2[:],
                                                         AF.Relu)
                                    nc.sync.dma_start(
                                        h_out.ap()[b * 128:(b + 1) * 128, :],
                                        hblk[:])
                        assert not psum_blk

                if layer < 3:
                    nc.gpsimd.collective_compute(
                        "AllGather", OP.bypass,
                        ins=[coll_in.opt()], outs=[coll_out.opt()],
                        replica_groups=[list(range(M))])
    nc.compile()
    return nc


# ---------------- public entry ----------------
_CACHE = {}


def _install_ntff_shim():
    import types
    if "antenv.axon_hooks" in sys.modules:
        return
    try:
        from trn_agent_boot.trn_boot import _ntff_profile_via_ctypes
        hook = _ntff_profile_via_ctypes("/opt/axon/libaxon_pjrt.so")
    except Exception:
        hook = None
    mod = types.ModuleType("antenv.axon_hooks")
    mod.get_axon_ntff_profile_hook = lambda: hook
    mod.set_axon_ntff_profile_hook = lambda h: None
    sys.modules["antenv.axon_hooks"] = mod
    try:
        import antenv
        antenv.axon_hooks = mod
    except Exception:
        pass


def run(inputs, cfg=SPEC, trace=False):
    d = _derive(cfg)
    wt = _host_weights(inputs, d)
    in_maps, meta = _prep(inputs["x"], inputs["edge_index"], d)
    for im in in_maps:
        im.update(wt)
    key = (tuple(sorted(cfg.items())), meta["TT"],
           tuple(tuple(r) for r in meta["tiles"]))
    if key not in _CACHE:
        _CACHE[key] = build_module(d, meta)
    nc = _CACHE[key]
    if trace:
        _install_ntff_shim()
    res = bass_utils.run_bass_kernel_spmd(
        nc, in_maps, core_ids=list(range(d["M"])), trace=trace)
    outs = [res.results[m]["h_out"][:d["NPC_REAL"]] for m in range(d["M"])]
    full = np.concatenate(outs, axis=0).astype(np.float32)
    return full, res


def kernel(**inputs) -> np.ndarray:
    trace = bool(os.environ.get("KERNEL_TRACE"))
    full, res = run(inputs, SPEC, trace=trace)
    if trace and res.exec_time_ns is not None:
        print(f"HW exec time: {res.exec_time_ns} ns")
    return full
